# revision 16
# baseline (speedup 1.0000x reference)
"""Trainium2 Bass kernel for nn_CPLinear (CP-decomposed QKV projection with RoPE).

Computes, for x:(2,4096,2048) and CP-factor weights:
    A_t = x @ W_A_t  (per-token head coefficients),  B_t = x @ W_B_t (shared bases)
    q = einsum('bshr,bsrd->bshd', A_q, rope(B_q)) / 12
    k = A_k * rope(B_k)   (rank-1)
    v = A_v * B_v         (rank-1)

Strategy (8 cores, data-parallel over the 8192 tokens, 1024 tokens/core):
  - All 6 projections fused into one [2048 x 2016] bf16 matmul; each k-chunk
    runs 4 matmuls off one stationary load (LDWEIGHTS fully amortized).
  - W/x stream in k-chunks split across both HWDGE queues; the first two token
    tiles run k-major, paced by chunk arrival, so the PE starts ~10us in.
  - PSUM: psa[512]x1 + psb[1536]x2 + psq[512]x1 = 8 banks; evictions ordered
    so no projection matmul ever waits on a PSUM release.
  - The rank-12 q contraction runs as block-diagonal matmuls (8 tokens per
    matmul, K=96); its 4 PSUM-bank groups are woven between later tiles'
    k-chunks so the scatter round-trip and qsb evictions hide under PE work.
  - q is written in raw block-diagonal layout and untangled on the host.
"""

import sys

for _p in ("/opt/trn_rl_repo",):
    if _p not in sys.path:
        sys.path.insert(0, _p)

import numpy as np
import ml_dtypes

BF16 = ml_dtypes.bfloat16

SH = 1024          # tokens per core
H = 2048           # hidden
KT = H // 128      # 16 k-tiles
NT = SH // 128     # 8 token tiles per core
NOUT = 2016        # fused projection output width
WX = NOUT + 256    # W chunk + stage-1 x sliver, merged for one-DMA-per-queue
WSPL = 1136        # k-chunk DMA column split between the two HWDGE queues
NH, HD, RQ = 16, 128, 12

_CACHE = {}


def make_nc():
    import concourse.bacc as bacc
    from concourse import mybir

    dt = mybir.dt

    nc = bacc.Bacc(
        "TRN2",
        target_bir_lowering=False,
        debug=False,
        enable_asserts=False,
        num_devices=8,
    )

    x_d = nc.dram_tensor("x", (H, SH), dt.bfloat16, kind="ExternalInput")  # pre-transposed host-side
    w_d = nc.dram_tensor("w", (KT, 128, WX), dt.bfloat16, kind="ExternalInput")
    cos_d = nc.dram_tensor("cosr", (SH, 64), dt.bfloat16, kind="ExternalInput")
    sin_d = nc.dram_tensor("sinr", (SH, 64), dt.bfloat16, kind="ExternalInput")
    q_d = nc.dram_tensor("q", (NT, 128, NH * HD), dt.bfloat16, kind="ExternalOutput")
    k_d = nc.dram_tensor("k", (SH, NH * HD), dt.bfloat16, kind="ExternalOutput")
    v_d = nc.dram_tensor("v", (SH, NH * HD), dt.bfloat16, kind="ExternalOutput")
    return nc, (x_d, w_d, cos_d, sin_d, q_d, k_d, v_d)


def build_body(nc, tc, tensors):
    from contextlib import ExitStack

    from concourse import mybir

    dt = mybir.dt
    x_d, w_d, cos_d, sin_d, q_d, k_d, v_d = tensors

    with ExitStack() as ctx:
        P = ctx.enter_context
        const_pool = P(tc.tile_pool(name="const", bufs=1))
        w_sb = const_pool.tile([128, KT * WX], dt.bfloat16, tag="w_sb")
        xT = const_pool.tile([128, KT * SH], dt.bfloat16, tag="xT")
        cos_sb = const_pool.tile([128, NT * 64], dt.bfloat16, tag="cos_sb")
        sin_sb = const_pool.tile([128, NT * 64], dt.bfloat16, tag="sin_sb")
        lhs_bufs = [
            const_pool.tile([128, 2048], dt.bfloat16, tag=f"lhs{i}", name=f"lhs{i}")
            for i in range(3)
        ]
        bdr_bufs = [
            const_pool.tile([128, 2048], dt.bfloat16, tag=f"bdr{i}", name=f"bdr{i}")
            for i in range(3)
        ]

        # ---- startup DMAs: k-chunked, split across both HWDGE queues ----
        nc.gpsimd.dma_start(
            out=cos_sb[:].rearrange("p (t n) -> p t n", t=NT),
            in_=cos_d[:].rearrange("(t p) n -> p t n", p=128),
        )
        nc.gpsimd.dma_start(
            out=sin_sb[:].rearrange("p (t n) -> p t n", t=NT),
            in_=sin_d[:].rearrange("(t p) n -> p t n", p=128),
        )
        for tl in lhs_bufs:
            nc.gpsimd.memset(tl[:], 0.0)
        # stage-1 stream: W with tiles 0-1's x sliver appended host-side, so
        # each HWDGE queue gets exactly one big-packet DMA per k-chunk
        for kk in range(KT):
            nc.scalar.dma_start(
                out=w_sb[:, kk * WX : kk * WX + WSPL],
                in_=w_d[kk][:, 0:WSPL],
            )
            nc.sync.dma_start(
                out=w_sb[:, kk * WX + WSPL : (kk + 1) * WX],
                in_=w_d[kk][:, WSPL:WX],
            )
        # the remaining x tokens stream per-tile behind the stage-1 stream
        xTv = xT[:].rearrange("p (k t) -> p k t", k=KT)
        xdv = x_d[:].rearrange("(k pr) t -> pr k t", k=KT)
        for p in range(2, NT):
            eng = nc.sync if p % 2 == 0 else nc.scalar
            eng.dma_start(
                out=xTv[:, :, p * 128 : (p + 1) * 128],
                in_=xdv[:, :, p * 128 : (p + 1) * 128],
            )

        psa_pool = P(tc.tile_pool(name="psa", bufs=1, space="PSUM"))
        psb_pool = P(tc.tile_pool(name="psb", bufs=2, space="PSUM"))
        psq_pool = P(tc.tile_pool(name="psq", bufs=1, space="PSUM"))
        small_pool = P(tc.tile_pool(name="small", bufs=3))
        bq_pool = P(tc.tile_pool(name="bq", bufs=2))
        bqr_pool = P(tc.tile_pool(name="bqr", bufs=2))
        rope_pool = P(tc.tile_pool(name="rope", bufs=3))
        out_pool = P(tc.tile_pool(name="outs", bufs=3))
        scr_pool = P(tc.tile_pool(name="scr", bufs=3, space="DRAM"))

        def proj_chunk(p, ps_a, ps_b, kk, parts="ab"):
            t0 = p * 128
            if p < 2:
                lh = w_sb[:, kk * WX + NOUT + t0 : kk * WX + NOUT + t0 + 128]
            else:
                lh = xT[:, kk * SH + t0 : kk * SH + t0 + 128]
            wb = kk * WX
            st = kk == 0
            sp = kk == KT - 1
            if "a" in parts:
                nc.tensor.matmul(
                    ps_a[:, 0:480], lh, w_sb[:, wb : wb + 480], start=st,
                    stop=sp,
                )
            if "b" in parts:
                for c in range(3):
                    nc.tensor.matmul(
                        ps_b[:, c * 512 : (c + 1) * 512],
                        lh,
                        w_sb[:, wb + 480 + c * 512 : wb + 480 + (c + 1) * 512],
                        start=st,
                        stop=sp,
                    )

        state = {}

        def post_a(p, ps_a):
            """psa eviction + A' bounce + ropeK + k/v for proj tile p."""
            t0 = p * 128
            scr = scr_pool.tile([128, 1728], dt.bfloat16, tag="scr",
                                name=f"scr{p}")
            smalls = small_pool.tile([128, 480], dt.bfloat16, tag="smalls")
            bkr = small_pool.tile([128, 128], dt.bfloat16, tag="bkr")
            tka = small_pool.tile([128, 64], dt.bfloat16, tag="tka")
            tkb = small_pool.tile([128, 64], dt.bfloat16, tag="tkb")
            nc.scalar.copy(smalls[:], ps_a[:, 0:480])
            # A' -> scratch (read back by l_v)
            nc.gpsimd.dma_start(out=scr[:, 1536:1728], in_=smalls[:, 0:192])
            state[p] = {"scr": scr, "smalls": smalls, "bkr": bkr,
                        "tka": tka, "tkb": tkb}

        def post_kv(p):
            """ropeK + k/v rank-1 broadcasts + outputs for proj tile p."""
            t0 = p * 128
            st = state[p]
            smalls, bkr, tka, tkb = (st["smalls"], st["bkr"], st["tka"],
                                     st["tkb"])
            cos_k = cos_sb[:, p * 64 : (p + 1) * 64]
            sin_k = sin_sb[:, p * 64 : (p + 1) * 64]
            bkv = smalls[:, 224:352].rearrange("p (two d) -> p two d", two=2)
            bkrv = bkr[:].rearrange("p (two d) -> p two d", two=2)
            nc.vector.tensor_mul(tka[:], bkv[:, 0], cos_k)
            nc.vector.tensor_mul(tkb[:], bkv[:, 1], sin_k)
            nc.vector.tensor_sub(bkrv[:, 0], tka[:], tkb[:])
            nc.vector.tensor_mul(tka[:], bkv[:, 1], cos_k)
            nc.vector.tensor_mul(tkb[:], bkv[:, 0], sin_k)
            nc.vector.tensor_add(bkrv[:, 1], tka[:], tkb[:])
            ksb = out_pool.tile([128, 2048], dt.bfloat16, tag="ksb")
            vsb = out_pool.tile([128, 2048], dt.bfloat16, tag="vsb")
            nc.vector.tensor_mul(
                ksb[:].rearrange("p (h d) -> p h d", h=NH),
                bkr[:].unsqueeze(1).broadcast_to([128, NH, 128]),
                smalls[:, 192:208].unsqueeze(2).broadcast_to([128, NH, 128]),
            )
            nc.vector.tensor_mul(
                vsb[:].rearrange("p (h d) -> p h d", h=NH),
                smalls[:, 352:480].unsqueeze(1).broadcast_to([128, NH, 128]),
                smalls[:, 208:224].unsqueeze(2).broadcast_to([128, NH, 128]),
            )
            nc.sync.dma_start(out=k_d[t0 : t0 + 128, :], in_=ksb[:])
            nc.sync.dma_start(out=v_d[t0 : t0 + 128, :], in_=vsb[:])

        def post_b(p, ps_b, fast_tail=False, do_lv=True):
            """psb eviction, rope on B_q, bounce + scatter reads for tile p."""
            st = state[p]
            scr = st["scr"]
            bq = bq_pool.tile([128, 1536], dt.bfloat16, tag="bq")
            bqr = bqr_pool.tile([128, 1536], dt.bfloat16, tag="bqr")
            ta = rope_pool.tile([128, 768], dt.bfloat16, tag="ta")
            tb = rope_pool.tile([128, 768], dt.bfloat16, tag="tb")
            nc.scalar.copy(bq[:], ps_b[:])
            cosr = (
                cos_sb[:, p * 64 : (p + 1) * 64]
                .unsqueeze(1)
                .broadcast_to([128, RQ, 64])
            )
            sinr = (
                sin_sb[:, p * 64 : (p + 1) * 64]
                .unsqueeze(1)
                .broadcast_to([128, RQ, 64])
            )
            sv = bq[:].rearrange("p (r two d) -> p r two d", r=RQ, two=2)
            dv = bqr[:].rearrange("p (r two d) -> p r two d", r=RQ, two=2)
            tav = ta[:].rearrange("p (r d) -> p r d", r=RQ)
            tbv = tb[:].rearrange("p (r d) -> p r d", r=RQ)
            p_lo = sv[:, :, 0]
            p_hi = sv[:, :, 1]
            nc.vector.tensor_mul(tav, p_lo, cosr)
            nc.vector.tensor_mul(tbv, p_hi, sinr)
            nc.vector.tensor_sub(dv[:, :, 0], tav, tbv)
            nc.vector.tensor_mul(tav, p_hi, cosr)
            nc.vector.tensor_mul(tbv, p_lo, sinr)
            nc.vector.tensor_add(dv[:, :, 1], tav, tbv)

            # bounce roped B_q, then scatter-read the block-diagonal operands
            w_bq_eng = nc.sync if fast_tail else nc.scalar
            w_bq_eng.dma_start(out=scr[:, 0:1536], in_=bqr[:])
            if do_lv:
                lv_reads(p)
            bdr = bdr_bufs[p % 3]
            sb_v = scr[:, 0:1536].rearrange("(g t) (r d) -> t r g d", t=8, r=RQ)
            d_v = bdr[0:96, :].rearrange("(t r) (g d) -> t r g d", t=8, g=16)
            for t in range(8):
                if fast_tail:
                    eng = (nc.sync, nc.scalar)[t % 2]
                elif p == NT - 2:
                    eng = nc.gpsimd if t in (2, 5, 7) else nc.sync
                else:
                    eng = (nc.sync, nc.scalar, nc.gpsimd)[t % 3]
                eng.dma_start(out=d_v[t], in_=sb_v[t])
            st["bdr"] = bdr

        def lv_reads(p, fast=False):
            scr = state[p]["scr"]
            lhs = lhs_bufs[p % 3]
            sa_v = scr[:, 1536:1728].rearrange(
                "(g t) (r h) -> t r g h", t=8, r=RQ
            )
            l_v = lhs[0:96, :].rearrange("(t r) (g c) -> t r g c", t=8, g=16)
            for t in range(8):
                eng = nc.sync if fast else nc.gpsimd
                eng.dma_start(
                    out=l_v[t][:, :, t * 16 : (t + 1) * 16], in_=sa_v[t]
                )
            state[p]["lhs"] = lhs

        def bd_group(p, gq, pool=None):
            """one PSUM-bank group (4 block-diagonal matmuls) of tile p's q."""
            st = state[p]
            if gq == 0:
                st["qsb"] = out_pool.tile([128, 2048], dt.bfloat16, tag="qsb",
                                          name=f"qsb{p}")
            lhs, bdr, qsb = st["lhs"], st["bdr"], st["qsb"]
            if pool is None:
                qp = psq_pool.tile([128, 512], dt.float32, tag="qp",
                                   name=f"qp{p}_{gq}")
            else:
                qp = pool.tile([128, 512], dt.float32, tag="psa",
                               name=f"qp{p}_{gq}")
            for j4 in range(4):
                g = gq * 4 + j4
                nc.tensor.matmul(
                    qp[:, j4 * 128 : (j4 + 1) * 128],
                    lhs[0:96, g * 128 : (g + 1) * 128],
                    bdr[0:96, g * 128 : (g + 1) * 128],
                    start=True,
                    stop=True,
                )
            nc.scalar.copy(qsb[:, gq * 512 : (gq + 1) * 512], qp[:])
            if gq == 3:
                nc.scalar.dma_start(out=q_d[p], in_=qsb[:])

        # ================= schedule =================
        # stage 1: proj tiles 0,1 k-major, paced by the chunked W/x DMAs.
        # tile 1's A-block accumulates in the (otherwise idle) psq pool.
        ps_a0 = psa_pool.tile([128, 512], dt.float32, tag="psa", name="psa0")
        ps_b0 = psb_pool.tile([128, 1536], dt.float32, tag="psb", name="psb0")
        ps_a1 = psq_pool.tile([128, 512], dt.float32, tag="qp", name="psa1")
        ps_b1 = psb_pool.tile([128, 1536], dt.float32, tag="psb", name="psb1")
        for kk in range(KT):
            proj_chunk(0, ps_a0, ps_b0, kk)
            proj_chunk(1, ps_a1, ps_b1, kk)
        post_a(0, ps_a0)
        post_b(0, ps_b0)
        post_a(1, ps_a1)
        post_b(1, ps_b1)
        post_kv(0)
        post_kv(1)

        # stage 2: tiles 2-7 tile-major with the BD contraction woven in.
        # proj(3) carries the BD(0)/BD(1) catch-up; from then on BD(p-1)
        # starts at post(p) and finishes inside proj(p+1).
        weave = {
            3: [(0, 0, 1), (0, 1, 3), (0, 2, 5), (0, 3, 7),
                (1, 0, 9), (1, 1, 11), (1, 2, 13), (1, 3, 15)],
        }
        for p in range(4, NT - 1):
            weave[p] = [(p - 2, 1, 3), (p - 2, 2, 7), (p - 2, 3, 11)]
        for p in range(2, NT - 1):
            ps_a = psa_pool.tile([128, 512], dt.float32, tag="psa",
                                 name=f"psa{p}")
            ps_b = psb_pool.tile([128, 1536], dt.float32, tag="psb",
                                 name=f"psb{p}")
            slots = {kk: (bp, gq) for (bp, gq, kk) in weave.get(p, [])}
            for kk in range(KT):
                proj_chunk(p, ps_a, ps_b, kk)
                if kk in slots:
                    bd_group(*slots[kk])
            post_a(p, ps_a)
            post_b(p, ps_b)
            if p >= 3:
                bd_group(p - 1, 0)
            post_kv(p)
        # last tile: A-block columns first so the A'/k/v chains run during
        # the B_q pass, which in turn ends early enough that the bounce
        # round-trip hides under the reserved BD groups
        p = NT - 1
        ps_a = psa_pool.tile([128, 512], dt.float32, tag="psa", name="psa7")
        ps_b = psb_pool.tile([128, 1536], dt.float32, tag="psb", name="psb7")
        for kk in range(KT):
            proj_chunk(p, ps_a, ps_b, kk, parts="a")
        post_a(p, ps_a)
        lv_reads(p, fast=True)
        post_kv(p)
        for kk in range(KT):
            proj_chunk(p, ps_a, ps_b, kk, parts="b")
        post_b(p, ps_b, fast_tail=True, do_lv=False)
        # tail: reserved BD groups ping-pong between the psq and freed psa
        # banks while tile 7's bounce chain completes
        tail_groups = ([(NT - 3, gq) for gq in range(1, 4)]
                       + [(NT - 2, gq) for gq in range(4)]
                       + [(NT - 1, gq) for gq in range(4)])
        for i, (bp, gq) in enumerate(tail_groups):
            bd_group(bp, gq, pool=(psa_pool if i % 2 == 1 else None))


def build_program():
    import concourse.tile as tile

    nc, tensors = make_nc()
    with tile.TileContext(nc) as tc:
        build_body(nc, tc, tensors)
    nc.compile()
    return nc


def _get_program():
    if "nc" not in _CACHE:
        _CACHE["nc"] = build_program()
    return _CACHE["nc"]


def make_in_maps(x, W_A_q, W_B_q, W_A_k, W_B_k, W_A_v, W_B_v):
    """Shard + preprocess full inputs into per-core input maps."""
    x = np.asarray(x)
    B, S, Hh = x.shape
    x2 = np.ascontiguousarray(x.reshape(B * S, Hh))

    # fold the 1/RQ scale and the (h,r)->(r,h) column reorder into W_A_q
    WAq = np.asarray(W_A_q).reshape(Hh, NH, RQ).transpose(0, 2, 1).reshape(
        Hh, NH * RQ
    ) / np.float32(RQ)
    Wall = np.concatenate(
        [
            WAq,
            np.asarray(W_A_k),
            np.asarray(W_A_v),
            np.asarray(W_B_k),
            np.asarray(W_B_v),
            np.asarray(W_B_q),
        ],
        axis=1,
    )
    assert Wall.shape == (Hh, NOUT)
    Wt = np.ascontiguousarray(Wall.reshape(KT, 128, NOUT)).astype(BF16)

    inv = 1.0 / (10000.0 ** (np.arange(0, HD, 2, dtype=np.float32) / HD))
    ang = np.arange(S, dtype=np.float32)[:, None] * inv[None, :]
    cos_rep = np.ascontiguousarray(np.cos(ang)).astype(BF16)
    sin_rep = np.ascontiguousarray(np.sin(ang)).astype(BF16)

    in_maps = []
    for i in range(8):
        tok0 = i * SH
        pos = np.arange(tok0, tok0 + SH) % S
        xt = np.ascontiguousarray(x2[tok0 : tok0 + SH].T).astype(BF16)
        x1 = xt[:, 0:256].reshape(KT, 128, 256)
        wx = np.ascontiguousarray(np.concatenate([Wt, x1], axis=2))
        in_maps.append(
            {
                # pre-transposed (hidden, tokens) so on-chip loads are plain
                "x": xt,
                "w": wx,
                "cosr": np.ascontiguousarray(cos_rep[pos]),
                "sinr": np.ascontiguousarray(sin_rep[pos]),
            }
        )
    return in_maps, (B, S)


def assemble_outputs(results, B, S):
    # q arrives in raw block-diagonal layout: [p, t*16+h, g*128+d] with
    # token = p*128 + g*8 + t
    qs = []
    for i in range(8):
        a = results[i]["q"].astype(np.float32).reshape(NT, 8, 16, 16, 128)
        qs.append(a.transpose(0, 3, 1, 2, 4).reshape(SH, NH, HD))
    q = np.concatenate(qs, axis=0).reshape(B, S, NH, HD)
    k = np.concatenate(
        [results[i]["k"].astype(np.float32) for i in range(8)], axis=0
    ).reshape(B, S, NH, HD)
    v = np.concatenate(
        [results[i]["v"].astype(np.float32) for i in range(8)], axis=0
    ).reshape(B, S, NH, HD)
    return q, k, v


def kernel(x, W_A_q, W_B_q, W_A_k, W_B_k, W_A_v, W_B_v):
    from concourse.bass_utils import run_bass_kernel_spmd

    nc = _get_program()
    in_maps, (B, S) = make_in_maps(x, W_A_q, W_B_q, W_A_k, W_B_k, W_A_v, W_B_v)
    res = run_bass_kernel_spmd(nc, in_maps, list(range(8))).results
    return assemble_outputs(res, B, S)
